# revision 21
# baseline (speedup 1.0000x reference)
"""Trainium2 Bass kernel for nn_Decoder_69337952026902 (3-layer CfC RNN decoder).

Math reformulation (validated to 2e-7 vs the jax reference):
  - CfC cell: h' = tanh(ff1)*(1-sig(ta+tb)) + sig(ta+tb)*tanh(ff2)
  - ta+tb folded into one matmul block u = (Wta+Wtb)x + (bta+btb); sigmoid via
    tanh: sig(u) = (1+tanh(u/2))/2 with the 1/2 folded into the u weights.
  - State kept doubled (s = 2h) so s' = (tu+1)*t2 - (tu-1)*t1 with
    t1,t2,tu = tanh of the three z blocks; the 1/2 is folded into all weight
    columns that consume a state.
  - Per-layer z = [ff1 | ff2 | u] (3h wide) computed as ONE matmul group.

Device mapping (per core, batch shard Bc=64):
  - z (Bc x 3h) in PSUM via accumulating matmuls: lhsT = transposed-state
    staging tiles in SBUF (K-dim on partitions), rhs = folded weights.
  - bf16 matmul path (KBF16=0 falls back to fp32/float32r): 1 cyc/row PE
    streaming, 2x DVE, half the DMA bytes. Measured rel err 3.1e-3 (tol 2e-2).
  - One tanh ACT op per layer half, 3 fused DVE ops for the combine,
    PE-transpose of the new state back into the staging tiles.
  - Output y_t = s2 @ (Wout/2).T + bout batched at the end from the s2
    history tile; flushed to DRAM in 512-col chunks.
"""
import sys

sys.path.insert(0, "/opt/trn_rl_repo")

import numpy as np
import concourse.bacc as bacc
import concourse.mybir as mybir
import concourse.tile as tile
from concourse.bass_utils import run_bass_kernel_spmd

# ---- problem dims (hardcoded per contract) ----
B, L, I = 512, 256, 64
H0, H1, H2 = 269, 179, 64
N_CORES = 8
BC = B // N_CORES  # 64

F32 = mybir.dt.float32
F32R = mybir.dt.float32r
BF16 = __import__("os").environ.get("KBF16", "1") == "1"
EDT = mybir.dt.bfloat16 if BF16 else F32R   # matmul-path dtype
WDT = mybir.dt.bfloat16 if BF16 else F32    # elementwise/transpose dtype
TANH = mybir.ActivationFunctionType.Tanh
ADD = mybir.AluOpType.add
SUB = mybir.AluOpType.subtract
MUL = mybir.AluOpType.mult

# L0 psum column permutation: [ff1a|ff2a|ua | ff1b|ff2b|ub] (a=units 0:128, b=128:269)
PERM0 = np.concatenate([
    np.arange(0, 128), np.arange(269, 397), np.arange(538, 666),
    np.arange(128, 269), np.arange(397, 538), np.arange(666, 807),
])

_CACHE = {}
TRACE = False       # test-harness hook: capture NTFF profile on next kernel() call
TRACE_DIR = None


def _build_nc(n_steps=L):
    nc = bacc.Bacc("TRN2", debug=False)

    def din(name, shape, dt_=EDT):
        return nc.dram_tensor(name, list(shape), dt_, kind="ExternalInput").ap()

    dtT = din("dtT", (n_steps * 64, 64))
    w0g0 = din("w0g0", (78, 808))
    w0g1 = din("w0g1", (128, 808))
    w0g2 = din("w0g2", (128, 808))
    w1g0 = din("w1g0", (128, 538))
    w1g1 = din("w1g1", (128, 538))
    w1g2 = din("w1g2", (13, 538))
    w1g3 = din("w1g3", (128, 538))
    w1g4 = din("w1g4", (52, 538))
    w2g0 = din("w2g0", (128, 256))
    w2g1 = din("w2g1", (52, 256))
    w2g2 = din("w2g2", (65, 256))
    wy = din("wy", (65, 64))
    i_xt = din("init_xt", (128, 64))
    i_st0 = din("init_st0", (128, 128))
    i_st1 = din("init_st1", (128, 128))
    i_st2 = din("init_st2", (128, 64))
    identD = din("ident", (64, 64), WDT)
    onesD = din("onesrow", (1, n_steps * 64))
    onesblkD = din("onesblk", (64, 32), WDT)
    yout = nc.dram_tensor("yout", [64, n_steps * 64], F32, kind="ExternalOutput").ap()

    with tile.TileContext(nc) as tc, \
         tc.tile_pool(name="persist", bufs=1) as pp, \
         tc.tile_pool(name="psum", bufs=1, space="PSUM") as psp, \
         tc.tile_pool(name="work", bufs=3) as wp, \
         tc.tile_pool(name="ypool", bufs=2) as yp:
        # ---- persistent SBUF tiles ----
        xt = pp.tile([128, 64], EDT, name="xt")      # [x_t.T(64); s0.T 256:269; ones]
        st0 = pp.tile([128, 128], EDT, name="st0")   # s0.T rows 0:128 | 128:256
        st1 = pp.tile([128, 128], EDT, name="st1")   # s1.T 0:128 | [128:179; ones; dead]
        # s2.T history: slot k holds s2.T of step k-1 (slot 0 = init); row 64 = ones
        s2h = pp.tile([65, (n_steps + 1) * 64], EDT, name="s2h")
        ident = pp.tile([64, 64], WDT, name="idents")
        ws = {}
        for nm, ap in (("w0g0", w0g0), ("w0g1", w0g1), ("w0g2", w0g2),
                       ("w1g0", w1g0), ("w1g1", w1g1), ("w1g2", w1g2),
                       ("w1g3", w1g3), ("w1g4", w1g4),
                       ("w2g0", w2g0), ("w2g1", w2g1), ("w2g2", w2g2),
                       ("wy", wy)):
            if nm == "w1g2":
                # must share base_partition=64 with its lhsT (xt[64:77])
                t = pp.tile([128, 538], EDT, name=nm + "s", tag=nm + "s")
                nc.sync.dma_start(t[64:77, :], ap[:])
            else:
                t = pp.tile(list(ap.shape), EDT, name=nm + "s", tag=nm + "s")
                nc.sync.dma_start(t[:], ap[:])
            ws[nm] = t

        # ---- persistent PSUM tiles (8 banks total) ----
        z0 = psp.tile([64, 1024], F32, name="z0")    # A@0:384, B@512:935
        z1 = psp.tile([64, 1024], F32, name="z1")    # 0:537
        z2 = psp.tile([64, 256], F32, name="z2")     # 0:192 real
        tp0 = psp.tile([128, 192], WDT, name="tp0")
        tp12 = psp.tile([128, 192], WDT, name="tp12")

        # ---- init ----
        nc.sync.dma_start(xt[:], i_xt[:])
        nc.sync.dma_start(st0[:], i_st0[:])
        nc.sync.dma_start(st1[:], i_st1[:])
        nc.sync.dma_start(ident[:], identD[:])
        # ones rows used by the merged st1 copy (cp45) every step: only the PE
        # can write PSUM, and bf16 PSUM memset is invalid ISA -> transpose a
        # ones block through the PE once at init.
        onesblk = pp.tile([64, 32], WDT, name="onesblk")
        nc.sync.dma_start(onesblk[:], onesblkD[:])
        nc.tensor.transpose(tp12[32:64, 64:128], onesblk[:], ident[:])
        nc.sync.dma_start(s2h[64:65, 64:], onesD[:])
        nc.sync.dma_start(s2h[0:65, 0:64], i_st2[0:65, :])

        # ---- software-pipelined step loop ----
        # Iteration t emits: L0(t) matmuls; L2(t-1) matmuls; L1(t-1) elementwise
        # tail; L1(t) old-state matmuls; L0(t) elementwise; L1(t) new-state
        # matmuls; L2(t-1) elementwise. The one-step skew keeps each in-order
        # engine stream free of long cross-engine latency stalls: step t+1's
        # L0 matmuls never sit behind step t's layer-2 tail.

        def dma_x(t):
            nc.sync.dma_start(xt[0:64, :], dtT[t * 64:(t + 1) * 64, :])

        def l0_mms(t):
            for zo, ra, rb in ((0, 0, 384), (512, 384, 808)):
                n = rb - ra
                nc.tensor.matmul(z0[:, zo:zo + n], st0[:, 0:64],
                                 ws["w0g1"][:, ra:rb], start=True, stop=False)
                nc.tensor.matmul(z0[:, zo:zo + n], st0[:, 64:128],
                                 ws["w0g2"][:, ra:rb], start=False, stop=False)
                nc.tensor.matmul(z0[:, zo:zo + n], xt[0:78, :],
                                 ws["w0g0"][:, ra:rb], start=False, stop=True)

        def l0_elem(t):
            t0a = wp.tile([64, 384], WDT, tag="t0a")
            # split tanh: ff2|tu cols first so the p0a combine starts earlier
            nc.scalar.activation(t0a[:, 128:384], z0[:, 128:384], TANH)
            nc.scalar.activation(t0a[:, 0:128], z0[:, 0:128], TANH)
            p0a = wp.tile([64, 128], WDT, tag="p0a")
            q0a = wp.tile([64, 128], WDT, tag="q0a")
            s0a = wp.tile([64, 128], WDT, tag="s0a")
            nc.vector.scalar_tensor_tensor(p0a[:], t0a[:, 256:384], 1.0,
                                           t0a[:, 128:256], ADD, MUL)
            nc.vector.scalar_tensor_tensor(q0a[:], t0a[:, 256:384], 1.0,
                                           t0a[:, 0:128], SUB, MUL)
            nc.vector.tensor_tensor(s0a[:], p0a[:], q0a[:], SUB)
            nc.tensor.transpose(tp0[:, 0:64], s0a[:], ident[:])
            nc.vector.tensor_copy(st0[:, 0:64], tp0[:, 0:64])
            t0b = wp.tile([64, 423], WDT, tag="t0b")
            # split tanh: ff2|tu cols first so the p0b combine starts earlier
            nc.scalar.activation(t0b[:, 141:423], z0[:, 653:935], TANH)
            nc.scalar.activation(t0b[:, 0:141], z0[:, 512:653], TANH)
            p0b = wp.tile([64, 141], WDT, tag="p0b")
            q0b = wp.tile([64, 141], WDT, tag="q0b")
            s0b = wp.tile([64, 141], WDT, tag="s0b")
            nc.vector.scalar_tensor_tensor(p0b[:], t0b[:, 282:423], 1.0,
                                           t0b[:, 141:282], ADD, MUL)
            nc.vector.scalar_tensor_tensor(q0b[:], t0b[:, 282:423], 1.0,
                                           t0b[:, 0:141], SUB, MUL)
            nc.vector.tensor_tensor(s0b[:], p0b[:], q0b[:], SUB)
            nc.tensor.transpose(tp0[:, 64:128], s0b[:, 0:128], ident[:])
            nc.tensor.transpose(tp0[0:13, 128:192], s0b[:, 128:141], ident[:])
            nc.vector.tensor_copy(st0[:, 64:128], tp0[:, 64:128])
            nc.vector.tensor_copy(xt[64:77, :], tp0[0:13, 128:192])

        def l1_old_mms(t):
            for ra, rb in ((0, 512), (512, 538)):
                nc.tensor.matmul(z1[:, ra:rb], st1[:, 0:64],
                                 ws["w1g3"][:, ra:rb], start=True, stop=False)
                nc.tensor.matmul(z1[:, ra:rb], st1[0:52, 64:128],
                                 ws["w1g4"][:, ra:rb], start=False, stop=False)

        def l1_new_mms(t):
            for ra, rb in ((0, 512), (512, 538)):
                nc.tensor.matmul(z1[:, ra:rb], st0[:, 0:64],
                                 ws["w1g0"][:, ra:rb], start=False, stop=False)
                nc.tensor.matmul(z1[:, ra:rb], st0[:, 64:128],
                                 ws["w1g1"][:, ra:rb], start=False, stop=False)
                nc.tensor.matmul(z1[:, ra:rb], xt[64:77, :],
                                 ws["w1g2"][64:77, ra:rb], start=False, stop=True)

        def l1_tail_act(t):
            # ACT + combine for layer 1 of step t. Emitted BEFORE step t+1's
            # L0 elementwise work: z1(t) is ready at step end, so this biases
            # the in-order ACT/DVE queues toward readiness order.
            t1b = wp.tile([64, 537], WDT, tag="t1b")
            # split tanh: ff2|tu cols first so the p1 combine starts earlier
            nc.scalar.activation(t1b[:, 179:537], z1[:, 179:537], TANH)
            nc.scalar.activation(t1b[:, 0:179], z1[:, 0:179], TANH)
            p1 = wp.tile([64, 179], WDT, tag="p1")
            q1 = wp.tile([64, 179], WDT, tag="q1")
            s1 = wp.tile([64, 179], WDT, tag="s1")
            nc.vector.scalar_tensor_tensor(p1[:], t1b[:, 358:537], 1.0,
                                           t1b[:, 179:358], ADD, MUL)
            nc.vector.scalar_tensor_tensor(q1[:], t1b[:, 358:537], 1.0,
                                           t1b[:, 0:179], SUB, MUL)
            nc.vector.tensor_tensor(s1[:], p1[:], q1[:], SUB)
            return s1

        def l1_tail_tp(t, s1):
            # transpose + staging copy (tensor-queue part kept at its old
            # emission point so PE order is unchanged)
            nc.tensor.transpose(tp12[:, 0:64], s1[:, 0:128], ident[:])
            nc.tensor.transpose(tp12[0:51, 64:128], s1[:, 128:179], ident[:])
            # merged copy; rows 51+ of cols 64:128 carry the preset ones row
            nc.vector.tensor_copy(st1[:, 0:128], tp12[:, 0:128])

        def l1_tail(t):
            l1_tail_tp(t, l1_tail_act(t))

        def l2_mms(t):
            nc.tensor.matmul(z2[:, 0:256], s2h[0:65, t * 64:(t + 1) * 64],
                             ws["w2g2"][:, 0:256], start=True, stop=False)
            nc.tensor.matmul(z2[:, 0:256], st1[:, 0:64],
                             ws["w2g0"][:, 0:256], start=False, stop=False)
            nc.tensor.matmul(z2[:, 0:256], st1[0:52, 64:128],
                             ws["w2g1"][:, 0:256], start=False, stop=True)

        def l2_elem(t):
            t2b = wp.tile([64, 192], WDT, tag="t2b")
            nc.scalar.activation(t2b[:], z2[:, 0:192], TANH)
            p2 = wp.tile([64, 64], WDT, tag="p2")
            q2 = wp.tile([64, 64], WDT, tag="q2")
            s2 = wp.tile([64, 64], WDT, tag="s2")
            nc.vector.scalar_tensor_tensor(p2[:], t2b[:, 128:192], 1.0,
                                           t2b[:, 64:128], ADD, MUL)
            nc.vector.scalar_tensor_tensor(q2[:], t2b[:, 128:192], 1.0,
                                           t2b[:, 0:64], SUB, MUL)
            nc.vector.tensor_tensor(s2[:], p2[:], q2[:], SUB)
            nc.tensor.transpose(tp12[0:64, 128:192], s2[:], ident[:])
            nc.vector.tensor_copy(s2h[0:64, (t + 1) * 64:(t + 2) * 64],
                                  tp12[0:64, 128:192])

        def y_phase():
            # Y.T = wy.T @ s2h[:, 64:]: one stationary weight, stream all steps.
            # z0 psum (2 banks) is free after the loop; alternate its halves.
            total = n_steps * 64
            offs = list(range(0, total, 512))
            for c, off in enumerate(offs):
                n = min(512, total - off)
                zoff = 512 * (c % 2)
                ysb = yp.tile([64, 512], F32, name="ysb", tag="ysb")
                nc.tensor.matmul(z0[:, zoff:zoff + n], ws["wy"][:],
                                 s2h[0:65, 64 + off:64 + off + n],
                                 start=True, stop=True)
                nc.vector.tensor_copy(ysb[:, 0:n], z0[:, zoff:zoff + n])
                nc.sync.dma_start(yout[:, off:off + n], ysb[:, 0:n])

        for t in range(n_steps):
            dma_x(t)
            s1c = l1_tail_act(t - 1) if t > 0 else None
            l0_mms(t)
            l0_elem(t)
            if t > 0:
                l1_tail_tp(t - 1, s1c)
                l2_mms(t - 1)
            l1_old_mms(t)
            l1_new_mms(t)
            if t > 0:
                l2_elem(t - 1)
        tl = n_steps - 1
        l1_tail(tl)
        l2_mms(tl)
        l2_elem(tl)
        y_phase()

    nc.compile()
    return nc


# ---------------- host-side weight folding ----------------

def _fold_layer(W, b, M, h, xw, scale_x):
    """Returns (rows, 3h) array: rows = [x dims; h dims; bias] in reformulated scaling."""
    W = np.asarray(W, np.float64)
    b = np.asarray(b, np.float64)
    M = np.abs(np.asarray(M, np.float64))
    Wff1 = W[0:h] * M
    Wff2 = W[h:2 * h] * M
    Wt = 0.5 * (W[2 * h:3 * h] + W[3 * h:4 * h])
    bcat = np.concatenate([b[0:h], b[h:2 * h], 0.5 * (b[2 * h:3 * h] + b[3 * h:4 * h])])
    Wcat = np.concatenate([Wff1, Wff2, Wt], 0)  # (3h, In)
    In = Wcat.shape[1]
    scale = np.ones(In)
    scale[xw:] = 0.5
    if scale_x:
        scale[:xw] = 0.5
    Wcat = Wcat * scale[None, :]
    full = np.concatenate([Wcat.T, bcat[None, :]], 0)  # (In+1, 3h)
    return full


def _prep_weights(inputs):
    f0 = _fold_layer(inputs["W0"], inputs["b0"], inputs["M0"], H0, I, False)   # (334, 807)
    f1 = _fold_layer(inputs["W1"], inputs["b1"], inputs["M1"], H1, H0, True)   # (449, 537)
    f2 = _fold_layer(inputs["W2"], inputs["b2"], inputs["M2"], H2, H1, True)   # (244, 192)
    f0 = f0[:, PERM0]
    f0 = np.concatenate([f0, np.zeros((334, 1))], 1)          # even-N pad (807->808)
    f1 = np.concatenate([f1, np.zeros((449, 1))], 1)          # even-N pad (537->538)
    x0, h0rows, b0row = f0[0:64], f0[64:333], f0[333:334]
    w = {}
    w["w0g0"] = np.concatenate([x0, h0rows[256:269], b0row], 0)
    w["w0g1"] = h0rows[0:128]
    w["w0g2"] = h0rows[128:256]
    h0r1, h1rows, b1row = f1[0:269], f1[269:448], f1[448:449]
    w["w1g0"] = h0r1[0:128]
    w["w1g1"] = h0r1[128:256]
    w["w1g2"] = h0r1[256:269]
    w["w1g3"] = h1rows[0:128]
    w["w1g4"] = np.concatenate([h1rows[128:179], b1row], 0)
    f2p = np.concatenate([f2, np.zeros((244, 64))], 1)  # pad cols to 256
    h1r2, h2rows, b2row = f2p[0:179], f2p[179:243], f2p[243:244]
    w["w2g0"] = h1r2[0:128]
    w["w2g1"] = np.concatenate([h1r2[128:179], b2row], 0)
    w["w2g2"] = np.concatenate([h2rows, np.zeros((1, 256))], 0)
    Wout = np.asarray(inputs["Wout"], np.float64)
    bout = np.asarray(inputs["bout"], np.float64)
    w["wy"] = np.concatenate([(0.5 * Wout).T, bout[None, :]], 0)
    npdt = mybir.dt.np(EDT)
    out = {k: np.ascontiguousarray(v.astype(np.float32), dtype=npdt) for k, v in w.items()}
    out["ident"] = np.ascontiguousarray(np.eye(64), dtype=mybir.dt.np(WDT))
    out["onesblk"] = np.ones((64, 32), dtype=mybir.dt.np(WDT))
    return out


def _prep_core_inputs(dt_shard, hs_shard, n_steps=L):
    """dt_shard (BC, L, I), hs_shard (BC, 512) -> per-core input arrays."""
    s = 2.0 * np.asarray(hs_shard, np.float64)
    s0T = np.ascontiguousarray(s[:, 0:H0].T)           # (269, 64)
    s1T = np.ascontiguousarray(s[:, H0:H0 + H1].T)     # (179, 64)
    s2T = np.ascontiguousarray(s[:, H0 + H1:].T)       # (64, 64)
    i_xt = np.zeros((128, 64))
    i_xt[64:77, :] = s0T[256:269]
    i_xt[77, :] = 1.0
    i_st0 = np.zeros((128, 128))
    i_st0[:, 0:64] = s0T[0:128]
    i_st0[:, 64:128] = s0T[128:256]
    i_st1 = np.zeros((128, 128))
    i_st1[:, 0:64] = s1T[0:128]
    i_st1[0:51, 64:128] = s1T[128:179]
    i_st1[51, 64:128] = 1.0
    i_st2 = np.zeros((128, 64))
    i_st2[0:64, :] = s2T
    i_st2[64, :] = 1.0
    dtT = np.asarray(dt_shard, np.float32)[:, :n_steps, :].transpose(1, 2, 0)  # (L, I, BC)
    dtT = np.ascontiguousarray(dtT.reshape(n_steps * 64, 64), dtype=np.float32)
    npdt = mybir.dt.np(EDT)
    return {
        "dtT": np.ascontiguousarray(dtT, npdt),
        "onesrow": np.ones((1, n_steps * 64), npdt),
        "init_xt": np.ascontiguousarray(i_xt, npdt),
        "init_st0": np.ascontiguousarray(i_st0, npdt),
        "init_st1": np.ascontiguousarray(i_st1, npdt),
        "init_st2": np.ascontiguousarray(i_st2, npdt),
    }


def kernel(dt, hidden_state, W0, b0, W1, b1, W2, b2, Wout, bout, M0, M1, M2,
           n_steps=L):
    inputs = dict(W0=W0, b0=b0, W1=W1, b1=b1, W2=W2, b2=b2,
                  Wout=Wout, bout=bout, M0=M0, M1=M1, M2=M2)
    w = _prep_weights(inputs)
    dt = np.asarray(dt, np.float32)
    hs = np.asarray(hidden_state, np.float32)
    in_maps = []
    for c in range(N_CORES):
        m = dict(w)
        m.update(_prep_core_inputs(dt[c * BC:(c + 1) * BC], hs[c * BC:(c + 1) * BC],
                                   n_steps))
        in_maps.append(m)
    key = ("nc", n_steps)
    if key not in _CACHE:
        _CACHE[key] = _build_nc(n_steps)
    nc = _CACHE[key]
    kwargs = {}
    if TRACE:
        kwargs = dict(trace=True, tmpdir=TRACE_DIR)
    res = run_bass_kernel_spmd(nc, in_maps, core_ids=list(range(N_CORES)), **kwargs)
    _CACHE["last_results"] = res
    out = np.empty((B, n_steps, 64), np.float32)
    for c in range(N_CORES):
        out[c * BC:(c + 1) * BC] = res.results[c]["yout"].reshape(64, n_steps, 64).transpose(2, 1, 0)
    return out
